# revision 36
# baseline (speedup 1.0000x reference)
"""Trainium2 Bass kernel for CDAttnBlock (v2 — pipelined).

Reference computation (per batch element b, all in fp32):
    q,k,v   = split(x  @ Wqkv)   heads=12, d=64
    q2,k2,v2= split(x2 @ Wqkv)
    o1 = attn(q, k,  v);  o2 = attn(q2, k2, v2);  o3 = attn(q, k2, v2)
    y_i = merge(o_i) @ Wout + bout

Sharding: pure data-parallel over batch (B=8) across 8 NeuronCores.

v2 design (vs v1 baseline at ~700us):
  - ScalarE exp stream (288 ACTs x ~1.15us = the metronome) starts ~30us
    in (right after x DMA + xT + pair-0 q/k priming) and never drains
    between attention phases: all other work (qkv of both inputs, x2
    transposes, output projections, softmax normalization) is pumped as
    small thunks into the per-step gaps.
  - Scores are computed per (pair, q-half, key-tile) with the two heads
    of a pair ROW-TILED onto disjoint PE quadrants (contraction d=64 at
    partitions 0-63 / 64-127) so both heads' score matmuls run
    concurrently; one [128,1024] PSUM tile holds [headA | headB] halves
    and one exp covers both.
  - av keeps the ones-column trick (lhsT [128, 65], row 64 = softmax
    denominator) accumulating over key-tiles into [65, 512] PSUM.
  - Normalize: denominators for all 12 heads collected into one
    [12, 1024] f16 tile -> ONE DVE reciprocal per attention (recip is
    ~6.4 cyc/elem so batching frees ~40us) -> gpsimd broadcast ->
    in-place f16 multiply on oT.
  - PSUM: scores 2x[128,1024] (4 banks) + ov 2x[65,512] (2) + aux 2x
    [128,512] (2) = 8 banks.
"""

import numpy as np

import concourse.bass as bass
import concourse.tile as tile
from concourse import bacc, mybir
from concourse.bass_utils import run_bass_kernel_spmd
from concourse.masks import make_identity

F32 = mybir.dt.float32
F16 = mybir.dt.float16
AF = mybir.ActivationFunctionType

HIDDEN = 768
HEADS = 12
D = 64
S = 1024
B = 8
SCALE = D ** -0.5
NPAIR = HEADS // 2          # 6 head pairs
KT = HIDDEN // 128          # 6 k-tiles over hidden
ST = S // 128               # 8 s-tiles
VW = D + 1                  # 65: v columns + ones column


class Ctx:
    """Shared handles for the kernel builder."""


# ---------------------------------------------------------------------------
# aux thunk builders (each returns a list of closures; every closure is a
# small burst of engine work suitable for pumping into exp-stream gaps)
# ---------------------------------------------------------------------------

def th_dma_x(c, x_ap, st, xns, key):
    nc = c.nc

    def f():
        xn = c.xnat.tile([128, HIDDEN], F32, name="xn", tag="xn")
        xns[(key, st)] = xn
        nc.sync.dma_start(xn[:], x_ap[st * 128:(st + 1) * 128, :])
    return [f]


def th_xt_dma(c, xT, st, xns, key):
    """x s-tile -> xT via DMA xbar transpose (frees PE): one f16 cast,
    then 6 [128,128] transposed DMAs into xT's kt-columns."""
    nc = c.nc
    out3 = xT.rearrange("p (h s) -> p h s", s=S)

    def f():
        x16 = c.xnat.tile([128, HIDDEN], F16, name="x16", tag="x16")
        nc.vector.tensor_copy(x16[:], xns[(key, st)][:])
        for kt in range(KT):
            nc.sync.dma_start(
                out3[:, kt, st * 128:(st + 1) * 128],
                x16[:, kt * 128:(kt + 1) * 128], transpose=True)
    return [f]


def th_xt(c, xT, st, xns, key):
    """PE-transpose one s-tile of x into xT [128, KT*S]; two thunks."""
    nc = c.nc
    out3 = xT.rearrange("p (h s) -> p h s", s=S)

    def tp(half):
        def f():
            pt = c.auxp.tile([128, 3 * 128], F32, name="tpp",
                             tag=c.aux_tag())
            for i in range(3):
                ht = 3 * half + i
                nc.tensor.transpose(
                    pt[:, i * 128:(i + 1) * 128],
                    xns[(key, st)][:, ht * 128:(ht + 1) * 128], c.ident[:])
            nc.vector.tensor_copy(
                out3[:, 3 * half:3 * half + 3, st * 128:(st + 1) * 128],
                pt.rearrange("p (h s) -> p h s", s=128))
        return f
    return [tp(0), tp(1)]


def th_v_half(c, xT, v_st, st, half):
    """v columns for heads [0..8) (half 0) or [8..12) (half 1) of s-tile."""
    nc = c.nc

    def f():
        lo, hi = (0, 512) if half == 0 else (512, 768)
        vp = c.auxp.tile([128, hi - lo], F32, name="vp", tag=c.aux_tag())
        for kt in range(KT):
            nc.tensor.matmul(
                vp[:], xT[:, kt * S + st * 128:kt * S + (st + 1) * 128],
                c.wq16[kt][:, 2 * HIDDEN + lo:2 * HIDDEN + hi],
                start=(kt == 0), stop=(kt == KT - 1))
        vs3 = v_st[st].rearrange("p (h w) -> p h w", w=VW)
        ha, hb = (0, 8) if half == 0 else (8, 12)
        nc.vector.tensor_copy(
            vs3[:, ha:hb, 0:D], vp.rearrange("p (h w) -> p h w", w=D))
    return [f]


def th_qk(c, xT, p, base, dst):
    """qT or kT for one pair: two halves x two 3-kt sub-thunks each
    (small bursts keep the exp stream fed)."""
    nc = c.nc

    def half(hf):
        hold = {}

        def fa():
            hold["pp"] = c.auxp.tile([128, 512], F32, name="qkp",
                                     tag=c.aux_tag())
            lo = hf * 512
            for kt in range(3):
                nc.tensor.matmul(
                    hold["pp"][:],
                    c.wq16[kt][:, base + p * 128:base + (p + 1) * 128],
                    xT[:, kt * S + lo:kt * S + lo + 512],
                    start=(kt == 0), stop=False)

        def fb():
            lo = hf * 512
            for kt in range(3, KT):
                nc.tensor.matmul(
                    hold["pp"][:],
                    c.wq16[kt][:, base + p * 128:base + (p + 1) * 128],
                    xT[:, kt * S + lo:kt * S + lo + 512],
                    start=False, stop=(kt == KT - 1))
            nc.vector.tensor_copy(dst[p][:, lo:lo + 512], hold["pp"][:])
        return [fa, fb]
    return half(0) + half(1)


def th_proj(c, oT, y_dram, st, psum_cycle=None):
    """y[st] = oT.T @ Wout + bias -> DRAM; two half-thunks.
    psum_cycle: optional callable yielding (pool, tag) — used in the
    tail to rotate over 4 free PSUM banks instead of the 2 aux banks."""
    nc = c.nc
    hold = {}

    def half(h):
        def f():
            lo, hi = (0, 512) if h == 0 else (512, 768)
            if psum_cycle is None:
                yp = c.auxp.tile([128, hi - lo], F32, name="yp",
                                 tag=c.aux_tag())
            else:
                pool, tag = psum_cycle()
                yp = pool.tile([128, hi - lo], F32, name="yp", tag=tag)
            for ct in range(KT):
                nc.tensor.matmul(
                    yp[:], oT[ct][:, st * 128:(st + 1) * 128],
                    c.wout16[ct][:, lo:hi],
                    start=(ct == 0), stop=(ct == KT - 1))
            if h == 0:
                hold["yt"] = c.ysb.tile([128, HIDDEN], F32, name="yt",
                                        tag="yt")
            yt = hold["yt"]
            nc.vector.tensor_add(yt[:, lo:hi], yp[:], c.bias_sb[:, lo:hi])
            if h == 1:
                nc.sync.dma_start(y_dram[st * 128:(st + 1) * 128, :], yt[:])
        return f
    return [half(0), half(1)]


def th_norm(c, oT, den4s, attn_id):
    """Deferred normalize chain for one attention (12 heads in 3 groups
    of 4): per group, upcast + fast-approx reciprocal + downcast, then
    per-head gpsimd bcast + in-place f16 multiply on oT."""
    nc = c.nc
    hold = {}

    def t_recip(g):
        def f():
            df = c.dnp.tile([128, S], F32, name="df", tag="df", bufs=1)
            nc.vector.tensor_copy(df[:], den4s[g][:])
            nc.vector.reciprocal_approx_fast(df[:], df[:])
            r16 = c.dnp.tile([128, S], F16, name="r16", tag=f"r16{g}",
                             bufs=1)
            nc.vector.tensor_copy(r16[:], df[:])
            hold[g] = r16
        return f

    def t_head(h):
        def f():
            p, hh = h // 2, h % 2
            hp = slice(hh * D, (hh + 1) * D)
            g, j = h // 4, h % 4
            # partition_broadcast only supports src/dst partition 0, so
            # stage the recip row down to partition 0, then broadcast to
            # all 128 and multiply against the matching half (tensor ops
            # need equal input base partitions).
            rrow = c.bcsp.tile([1, S], F16, name="rrow", tag="rrow",
                               bufs=1)
            nc.vector.tensor_copy(rrow[:], hold[g][32 * j:32 * j + 1, :])
            bcs = c.bcsp.tile([128, S], F16, name="bcs", tag="bcs",
                              bufs=2)
            nc.gpsimd.partition_broadcast(bcs[:], rrow[:])
            nc.vector.tensor_mul(oT[p][hp, :], oT[p][hp, :], bcs[hp, :])
        return f

    out = []
    for g in range(3):
        out.append(t_recip(g))
        out += [t_head(4 * g + j) for j in range(4)]
    return out


# ---------------------------------------------------------------------------
# the attention pipeline
# ---------------------------------------------------------------------------

def attention(c, qT, kT, v_st, oT, den4s, work, first_inline=None,
              at_site=None):
    """One attention (12 heads as 6 row-tiled pairs x 2 q-halves x 8
    key-tiles).

    `work` = list of (need_site, thunk): thunks are pumped into the
    exp-stream gaps at an even pace, but a thunk is ALWAYS emitted
    before the sweep whose site index reaches its need_site (program
    order on each engine queue is the dependency order -- a consumer
    emitted before its producer reads garbage).
    `first_inline`: 8 thunks run inside the first sweep, thunk[kt]
    right after exp(kt) and before av(kt) (used to produce v just in
    time for the very first attention)."""
    nc = c.nc
    state = {"i": 0, "credit": 0.0}
    sites = NPAIR * 2 * ST
    rate = max(0.001, len(work) / sites)

    def pump(k=1.0):
        state["credit"] += k * rate
        while state["credit"] >= 1.0 and state["i"] < len(work):
            work[state["i"]][1]()
            state["i"] += 1
            state["credit"] -= 1.0

    def force(site):
        while state["i"] < len(work) and work[state["i"]][0] <= site:
            work[state["i"]][1]()
            state["i"] += 1

    for pair in range(NPAIR):
        for qh in range(2):
            site = (pair * 2 + qh) * ST
            force(site)
            if at_site is not None:
                for f in at_site.pop(site, []):
                    f()
            qsl = slice(qh * 512, (qh + 1) * 512)
            ovA = c.ovps.tile([VW, 512], F32, name="ovA", tag="ovA")
            ovB = c.ovps.tile([VW, 512], F32, name="ovB", tag="ovB")
            for kt in range(ST):
                sp = c.sps.tile([128, S], F32, name="sp", tag="sp")
                ksl = slice(kt * 128, (kt + 1) * 128)
                nc.tensor.matmul(sp[:, 0:512], kT[pair][0:D, ksl],
                                 qT[pair][0:D, qsl], start=True, stop=True)
                nc.tensor.matmul(sp[:, 512:1024], kT[pair][D:128, ksl],
                                 qT[pair][D:128, qsl], start=True, stop=True)
                ex = c.exps.tile([128, S], F16, name="ex", tag="ex")
                nc.scalar.activation(ex[:], sp[:], AF.Exp,
                                     bias=c.zbias[:], scale=SCALE)
                if first_inline is not None and pair == 0 and qh == 0:
                    first_inline[kt]()
                vs3 = v_st[kt].rearrange("q (h w) -> q h w", w=VW)
                nc.tensor.matmul(ovA[:], vs3[:, 2 * pair, :], ex[:, 0:512],
                                 start=(kt == 0), stop=(kt == ST - 1))
                nc.tensor.matmul(ovB[:], vs3[:, 2 * pair + 1, :],
                                 ex[:, 512:1024],
                                 start=(kt == 0), stop=(kt == ST - 1))
                pump(1.0)
            # sweep tail: evacuate o (f16, pre-normalize) + denominators
            # (den row h goes to partition 32*(h%4) of group tile h//4 —
            # DVE moves must keep partition start congruent mod 32)
            hA, hB = 2 * pair, 2 * pair + 1
            nc.vector.tensor_copy(oT[pair][0:D, qsl], ovA[0:D, :])
            nc.vector.tensor_copy(
                den4s[hA // 4][32 * (hA % 4):32 * (hA % 4) + 1, qsl],
                ovA[D:VW, :])
            nc.vector.tensor_copy(oT[pair][D:128, qsl], ovB[0:D, :])
            nc.vector.tensor_copy(
                den4s[hB // 4][32 * (hB % 4):32 * (hB % 4) + 1, qsl],
                ovB[D:VW, :])
    # drain leftovers
    while state["i"] < len(work):
        work[state["i"]][1]()
        state["i"] += 1


def build_kernel(ctx, tc, x, x2, wq, wo, bo, y1, y2, y3):
    nc = tc.nc
    c = Ctx()
    c.nc = nc
    c._aux_flip = [0]

    def aux_tag():
        c._aux_flip[0] ^= 1
        return ("auxA", "auxB")[c._aux_flip[0]]
    c.aux_tag = aux_tag

    # ---------------- constants ---------------------------------------
    const = ctx.enter_context(tc.tile_pool(name="const", bufs=1))
    c.ident = const.tile([128, 128], F32, name="ident")
    make_identity(nc, c.ident)
    c.zbias = const.tile([128, 1], F32, name="zbias")
    nc.vector.memset(c.zbias[:], 0.0)
    c.onescol = const.tile([128, 1], F32, name="onescol")
    nc.vector.memset(c.onescol[:], 1.0)
    c.bias_sb = const.tile([128, HIDDEN], F32, name="bias_sb")

    # ---------------- persistent pools --------------------------------
    woutp = ctx.enter_context(tc.tile_pool(name="woutp", bufs=1))
    c.wout16 = [woutp.tile([128, HIDDEN], F16, name=f"wout{ct}",
                           tag=f"wout{ct}") for ct in range(KT)]
    qxp = ctx.enter_context(tc.tile_pool(name="qxp", bufs=1))
    qT_x = [qxp.tile([128, S], F16, name=f"qTx{i}", tag=f"qTx{i}")
            for i in range(NPAIR)]
    kvx2p = ctx.enter_context(tc.tile_pool(name="kvx2p", bufs=1))
    qT_x2 = [kvx2p.tile([128, S], F16, name=f"qTx2{i}", tag=f"qTx2{i}")
             for i in range(NPAIR)]
    kT_x2 = [kvx2p.tile([128, S], F16, name=f"kTx2{i}", tag=f"kTx2{i}")
             for i in range(NPAIR)]
    v_x2 = [kvx2p.tile([128, HEADS * VW], F16, name=f"vx2{i}",
                       tag=f"vx2{i}") for i in range(ST)]
    otp = ctx.enter_context(tc.tile_pool(name="otp", bufs=1))
    oT1 = [otp.tile([128, S], F16, name=f"oTa{i}", tag=f"oTa{i}")
           for i in range(NPAIR)]

    # pools released mid-build (allocated after the persistent ones)
    x2tp = tc.alloc_tile_pool(name="x2tp", bufs=1)
    x2T = x2tp.tile([128, KT * S], F16, name="x2T")
    kvxp = tc.alloc_tile_pool(name="kvxp", bufs=1)
    kT_x = [kvxp.tile([128, S], F16, name=f"kTx{i}", tag=f"kTx{i}")
            for i in range(NPAIR)]
    v_x = [kvxp.tile([128, HEADS * VW], F16, name=f"vx{i}", tag=f"vx{i}")
           for i in range(ST)]
    wqp = tc.alloc_tile_pool(name="wqp", bufs=1)
    c.wq16 = [wqp.tile([128, 3 * HIDDEN], F16, name=f"wq16{kt}",
                       tag=f"wq16{kt}") for kt in range(KT)]
    xtp = tc.alloc_tile_pool(name="xtp", bufs=1)
    xT = xtp.tile([128, KT * S], F16, name="xT")

    # ---------------- working pools (right side) ----------------------
    c.xnat = tc.alloc_tile_pool(name="xnat", bufs=2, side="right")
    c.exps = tc.alloc_tile_pool(name="exps", bufs=6, side="right")
    c.dnp = tc.alloc_tile_pool(name="dnp", bufs=2, side="right")
    c.bcsp = tc.alloc_tile_pool(name="bcsp", bufs=1, side="right")
    wstage = tc.alloc_tile_pool(name="wstage", bufs=2, side="right")

    # ---------------- PSUM pools --------------------------------------
    c.sps = tc.alloc_tile_pool(name="sps", bufs=2, space="PSUM")
    c.ovps = tc.alloc_tile_pool(name="ovps", bufs=1, space="PSUM")
    c.auxp = tc.alloc_tile_pool(name="auxp", bufs=1, space="PSUM")

    # ---------------- lead-in: DMA order + priming --------------------
    # x first (gates everything), then Wqkv, then Wout/bias, then x2.
    xns = {}
    for st in range(ST):
        th_dma_x(c, x, st, xns, "x")[0]()
    for hh in (0, 1, 2):
        for kt in range(KT):
            f = wstage.tile([128, HIDDEN], F32, name="wqf", tag="wqf")
            nc.sync.dma_start(
                f[:], wq[kt * 128:(kt + 1) * 128,
                         hh * HIDDEN:(hh + 1) * HIDDEN])
            nc.scalar.copy(
                c.wq16[kt][:, hh * HIDDEN:(hh + 1) * HIDDEN], f[:])
    bo_bcast = bass.AP(tensor=bo.tensor, offset=bo.offset,
                       ap=[[0, 128]] + list(bo.ap))
    nc.sync.dma_start(c.bias_sb[:], bo_bcast)

    # transposes of x (PE) as tiles arrive
    for st in range(ST):
        for f in th_xt(c, xT, st, xns, "x"):
            f()
    # prime pair-0 k and q (both halves) so the exp stream can start
    for f in th_qk(c, xT, 0, HIDDEN, kT_x) + th_qk(c, xT, 0, 0, qT_x):
        f()
    # ones columns of both v tensors (read by every av matmul)
    for vset in (v_x, v_x2):
        for st in range(ST):
            vs3 = vset[st].rearrange("p (h w) -> p h w", w=VW)
            nc.vector.memset(vs3[:, :, D:VW], 1.0)

    # stage Wout + x2 DMAs (issued now; consumed by aux thunks later)
    for ct in range(KT):
        f = wstage.tile([128, HIDDEN], F32, name="wof", tag="wqf")
        nc.sync.dma_start(f[:], wo[ct * 128:(ct + 1) * 128, :])
        nc.vector.tensor_copy(c.wout16[ct][:], f[:])
    for st in range(ST):
        th_dma_x(c, x2, st, xns, "x2")[0]()
    wstage.release()
    c.ysb = tc.alloc_tile_pool(name="ysb", bufs=2, side="right")

    def den_tiles():
        ts = [c.dnp.tile([128, S], F16, name="den4", tag=f"den4{g}",
                         bufs=1) for g in range(3)]
        for t in ts:
            nc.vector.memset(t[:], 1.0)
        return ts

    den_1 = den_tiles()

    # ---------------- attn1 = attn(q, k, v) ---------------------------
    # work items = (need_site, thunk): emitted before the sweep whose
    # site index reaches need_site; END = after this attention's sweeps
    # (the final drain), i.e. only read by a later phase.
    END = NPAIR * 2 * ST + 1
    inline_v = [th_v_half(c, xT, v_x, st, 0)[0] for st in range(ST)]
    work1 = []
    for p in range(1, NPAIR):
        dl = p * 2 * ST
        for f in th_qk(c, xT, p, HIDDEN, kT_x) + th_qk(c, xT, p, 0, qT_x):
            work1.append((dl, f))
    for st in range(ST):
        work1.append((4 * 2 * ST, th_v_half(c, xT, v_x, st, 1)[0]))
    for st in range(ST):
        for f in th_xt(c, x2T, st, xns, "x2"):
            work1.append((END, f))
    for p in range(NPAIR):
        for f in th_qk(c, x2T, p, HIDDEN, kT_x2):
            work1.append((END, f))
    for st in range(ST):
        work1.append((END, th_v_half(c, x2T, v_x2, st, 0)[0]))
        work1.append((END, th_v_half(c, x2T, v_x2, st, 1)[0]))
    work1.sort(key=lambda t: t[0])
    attention(c, qT_x, kT_x, v_x, oT1, den_1, work1,
              first_inline=inline_v)

    # ---------------- attn3 = attn(q, k2, v2) -------------------------
    oT3 = [otp.tile([128, S], F16, name=f"oTb{i}", tag=f"oTb{i}")
           for i in range(NPAIR)]
    den_3 = den_tiles()
    work3 = []
    for f in th_norm(c, oT1, den_1, 1):
        work3.append((END, f))
    for p in range(NPAIR):
        for f in th_qk(c, x2T, p, 0, qT_x2):
            work3.append((END, f))
    for st in range(ST):
        for f in th_proj(c, oT1, y1, st):
            work3.append((END, f))
    attention(c, qT_x, kT_x2, v_x2, oT3, den_3, work3)
    xtp.release()
    wqp.release()
    kvxp.release()

    # ---------------- attn2 = attn(q2, k2, v2) ------------------------
    oT2 = [otp.tile([128, S], F16, name=f"oTa{i}", tag=f"oTa{i}")
           for i in range(NPAIR)]
    den_2 = den_tiles()
    work2 = []
    for f in th_norm(c, oT3, den_3, 3):
        work2.append((END, f))
    for st in range(ST):
        for f in th_proj(c, oT3, y3, st):
            work2.append((END, f))
    # attn2 self-normalizes groups 0/1 mid-flight: den4 group g is fully
    # collected after pair (2g+1)'s qh1 sweep, so run its chain at the
    # start of sweep site (2g+2)*2*ST.
    norm2 = th_norm(c, oT2, den_2, 2)
    at2 = {(2 * 0 + 2) * 2 * ST: norm2[0:5],
           (2 * 1 + 2) * 2 * ST: norm2[5:10]}
    attention(c, qT_x2, kT_x2, v_x2, oT2, den_2, work2, at_site=at2)
    x2tp.release()

    # ---------------- tail: normalize group 2 + proj y2 ---------------
    for f in norm2[10:]:
        f()
    cyc = [(c.auxp, "auxA"), (c.auxp, "auxB"),
           (c.ovps, "ovA"), (c.ovps, "ovB")]
    cst = [0]

    def psum_cycle():
        cst[0] = (cst[0] + 1) % 4
        return cyc[cst[0]]

    for st in range(ST):
        for f in th_proj(c, oT2, y2, st, psum_cycle=psum_cycle):
            f()

    c.ysb.release()
    c.bcsp.release()
    c.dnp.release()
    c.exps.release()
    c.xnat.release()
    c.auxp.release()
    c.ovps.release()
    c.sps.release()


def build_bass():
    from contextlib import ExitStack
    nc = bacc.Bacc("TRN2", target_bir_lowering=False, debug=False,
                   num_devices=B)
    x = nc.dram_tensor("x", [S, HIDDEN], F32, kind="ExternalInput").ap()
    x2 = nc.dram_tensor("x2", [S, HIDDEN], F32, kind="ExternalInput").ap()
    wq = nc.dram_tensor("Wqkv", [HIDDEN, 3 * HIDDEN], F32,
                        kind="ExternalInput").ap()
    wo = nc.dram_tensor("Wout", [HIDDEN, HIDDEN], F32,
                        kind="ExternalInput").ap()
    bo = nc.dram_tensor("bout", [HIDDEN], F32, kind="ExternalInput").ap()
    y1 = nc.dram_tensor("y1", [S, HIDDEN], F32, kind="ExternalOutput").ap()
    y2 = nc.dram_tensor("y2", [S, HIDDEN], F32, kind="ExternalOutput").ap()
    y3 = nc.dram_tensor("y3", [S, HIDDEN], F32, kind="ExternalOutput").ap()
    with tile.TileContext(nc) as tc:
        with ExitStack() as ctx:
            build_kernel(ctx, tc, x, x2, wq, wo, bo, y1, y2, y3)
    nc.compile()
    return nc


_NC_CACHE = []


def kernel(x, x2, Wqkv, Wout, bout):
    if not _NC_CACHE:
        _NC_CACHE.append(build_bass())
    nc = _NC_CACHE[0]
    in_maps = [
        {"x": np.ascontiguousarray(x[b]), "x2": np.ascontiguousarray(x2[b]),
         "Wqkv": Wqkv, "Wout": Wout, "bout": bout}
        for b in range(B)
    ]
    res = run_bass_kernel_spmd(nc, in_maps, list(range(B)))
    y1 = np.stack([res.results[b]["y1"] for b in range(B)])
    y2 = np.stack([res.results[b]["y2"] for b in range(B)])
    y3 = np.stack([res.results[b]["y3"] for b in range(B)])
    return (y1, y2, y3)
